# revision 18
# baseline (speedup 1.0000x reference)
"""T5-style multi-head attention on 8 Trainium2 NeuronCores.

Problem: B=2, S=2048, D=1024, H=16 heads of 64; T5 relative-position bias
(32 buckets, max_distance=128), key mask, softmax, context.

Sharding: data-parallel over B (2) x tensor-parallel over head-groups of 4
(4 groups) = 8 cores.  Each core computes Q/K/V projections for its batch
and its 4 heads, then full attention for those heads.

v2: bf16 inputs/weights/activations (halves the input-DMA volume and
enables fast-weight-load), column-streamed projections software-pipelined
with the attention phase so the ACT engine (exp is the per-core compute
floor at ~126us) starts ~20us in instead of after all projections.

Device algorithm (per core), matmul cycles at 1 col/cycle in bf16:
  stream order: xk/xq/xv column-chunks of 1024; projections consume each
  chunk as it lands (Q/K as (X W)^T in [f, s] layout, V in [s, d] layout
  with a ones column for the softmax denominator).
  attention per (head, q2 chunk of 1024, k block of 128):
    scoresT[k,q] = K^T.T Q^T   (contraction d=64)
    expS = exp(scoresT + c_maj - 32) on ACT (c_maj = saturated-bucket bias)
    band fix (DVE) + minority saturated side (GPSIMD) as multiplicative
    corrections; ctxT[d|1,q] += V_ext.T expS accumulated over k blocks;
    row 0 of ctxT = softmax denominators (ones column rides along free).
  tail: reciprocal (DVE), partition broadcast (GPSIMD), scale, DMA out.

The first head's first-half k blocks are emitted before the second half
of the K/V streams arrive so ACT has work during the stream tail; its es
tiles for head h1 are held in SBUF until V lands.
"""

import numpy as np

import concourse.bacc as bacc
import concourse.tile as tile
from concourse import mybir
from concourse.bass_utils import run_bass_kernel_spmd

# problem dims (hardcoded per contract)
B = 2
S = 2048
DM = 1024
H = 16
HD = 64
NB = 32
MAXD = 128

HPC = 4          # heads per core
NCORES = 8
NDT = DM // 128  # 8 contraction tiles
NKB = S // 128   # 16 k blocks
NQ2 = 4          # q chunks of 512
QW = 512         # q chunk width
CW = 1024        # input stream chunk width
EBW = 384        # band table width

F32 = mybir.dt.float32
F32R = mybir.dt.float32r
BF16 = mybir.dt.bfloat16
F16 = mybir.dt.float16


def _rel_buckets():
    """T5 bidirectional bucket for rel = k - q in [-(S-1), S-1], fp32 math."""
    rel = np.arange(-(S - 1), S, dtype=np.int64)
    nb = NB // 2
    ret = (rel > 0).astype(np.int64) * nb
    rp = np.abs(rel)
    max_exact = nb // 2
    is_small = rp < max_exact
    rp_f = np.maximum(rp, 1).astype(np.float32)
    val = np.log(rp_f / np.float32(max_exact)) / np.float32(
        np.log(MAXD / max_exact)
    ) * np.float32(nb - max_exact)
    # XLA CPU f32->s32 convert rounds to nearest (cvtps2dq), not truncates
    val_large = max_exact + np.rint(val).astype(np.int32)
    val_large = np.minimum(val_large, nb - 1)
    return (ret + np.where(is_small, rp, val_large)).astype(np.int64)  # [2S-1]


def _band_bounds(kb):
    """Columns [a,b) of the non-saturated diagonal band for k block kb."""
    a = max(0, (kb - 1) * 128)
    b = min(S, (kb + 2) * 128)
    return a, b


def _maj_side(kb, q2):
    """Majority saturated side for (k block, q chunk): 0 -> bucket31 (q<a),
    1 -> bucket15 (q>=b)."""
    qlo, qhi = q2 * QW, (q2 + 1) * QW
    a, b = _band_bounds(kb)
    len31 = max(0, min(qhi, a) - qlo)
    len15 = max(0, qhi - max(qlo, b))
    return 0 if len31 >= len15 else 1


def build_program(use_mask, reps=1):
    nc = bacc.Bacc("TRN2", target_bir_lowering=False, debug=False,
                   num_devices=NCORES)

    xv = nc.dram_tensor("xv", [DM, S], F16, kind="ExternalInput").ap()
    xq = nc.dram_tensor("xq", [DM, S], F16, kind="ExternalInput").ap()
    xk = nc.dram_tensor("xk", [DM, S], F16, kind="ExternalInput").ap()
    wq = nc.dram_tensor("wq", [DM, HPC * HD], F16, kind="ExternalInput").ap()
    wk = nc.dram_tensor("wk", [DM, HPC * HD], F16, kind="ExternalInput").ap()
    wv = nc.dram_tensor("wv", [DM, HPC * HD], F16, kind="ExternalInput").ap()
    # band tables exp(g_h(rel) - c_maj): [side, head, 128, EBW]
    ebt = nc.dram_tensor("ebt", [2, HPC, 128, EBW], BF16,
                         kind="ExternalInput").ap()
    # per-(side, head): exp bias constant c_maj and minority ratio
    # cvals[0, side, h] = c_maj - 32 ; cvals[1, side, h] = exp(c_min - c_maj)
    cvals = nc.dram_tensor("cvals", [128, 2, 2, HPC], F32,
                           kind="ExternalInput").ap()
    vones = nc.dram_tensor("vones", [128, HPC * NKB], BF16,
                           kind="ExternalInput").ap()
    if use_mask:
        # additive mask term -1e4*(1-mask) laid out [128, NKB]
        mvals = nc.dram_tensor("mvals", [128, NKB], F32,
                               kind="ExternalInput").ap()
    # row 0 = softmax denominators, rows 1..64 = unnormalized context;
    # the division happens host-side during unsharding
    outp = nc.dram_tensor("out", [HPC, HD + 1, S], F32,
                          kind="ExternalOutput").ap()

    with tile.TileContext(nc) as tc:
        with tc.tile_pool(name="const", bufs=1) as const, \
             tc.tile_pool(name="qkt", bufs=1) as qkt, \
             tc.tile_pool(name="xs", bufs=3) as xs, \
             tc.tile_pool(name="esp", bufs=80) as esp, \
             tc.tile_pool(name="stgp", bufs=3) as stgp:

            # ---- resident constants ----
            w_sb = {}
            for nm, src in (("wk", wk), ("wq", wq), ("wv", wv)):
                t = const.tile([128, NDT, HPC * HD], F16, tag=nm, name=nm)
                nc.gpsimd.dma_start(
                    out=t[:], in_=src.rearrange("(dt p) f -> p dt f", p=128))
                w_sb[nm] = t
            cb = const.tile([128, 2, 2, HPC], F32, tag="cb", name="cb")
            nc.gpsimd.dma_start(out=cb[:], in_=cvals[:])
            eb_sb = const.tile([128, 2, HPC, EBW], BF16, tag="eb", name="eb")
            nc.gpsimd.dma_start(out=eb_sb[:],
                                in_=ebt.rearrange("m h p w -> p m h w"))
            if use_mask:
                mk = const.tile([128, NKB], F32, tag="mk", name="mk")
                nc.gpsimd.dma_start(out=mk[:], in_=mvals[:])

            # Q^T/K^T per pair: [128(2 heads x 64d), S] bf16
            qt = [qkt.tile([128, S], F16, tag=f"qt{p}", name=f"qt{p}")
                  for p in range(2)]
            kt = [qkt.tile([128, S], F16, tag=f"kt{p}", name=f"kt{p}")
                  for p in range(2)]
            # V_ext: [128(k in block), head, kblock, 65(1|d)]
            vx = qkt.tile([128, HPC, NKB, HD + 1], BF16, tag="vx", name="vx")
            nc.gpsimd.dma_start(
                out=vx[:, :, :, 0:1],
                in_=vones.rearrange("p (h k one) -> p h k one", h=HPC, one=1))

            for _rep in range(reps):
              # psum pools ([*,512] f32 = 1 bank each):
              # sps 4 + ctx 2 + pp 2 = 8 banks
              with tc.tile_pool(name="pp", bufs=2, space="PSUM") as pp, \
                   tc.tile_pool(name="spsp", bufs=4, space="PSUM") as spsp, \
                   tc.tile_pool(name="ctxp", bufs=2, space="PSUM") as ctxp:

                def stream_chunk(src, tag, c2, q=None):
                    """One [dm, 1024]-column chunk as a single DMA into a
                    [128, dt, 1024] tile (one issue, 2KB descriptors)."""
                    q = q or nc.sync
                    t = xs.tile([128, NDT, CW], F16, tag="x", name=f"x{tag}")
                    q.dma_start(
                        out=t[:],
                        in_=src[:, c2 * CW:(c2 + 1) * CW].rearrange(
                            "(dt p) w -> p dt w", p=128))
                    return t

                def qk_pass(wname, dst, xt, c2, fb, sc):
                    """One 512-col psum pass of (X W)^T."""
                    ps = pp.tile([128, 512], F32, tag="pp", name="pj")
                    for dt in range(NDT):
                        nc.tensor.matmul(
                            ps[:],
                            lhsT=w_sb[wname][:, dt,
                                             fb * 128:(fb + 1) * 128],
                            rhs=xt[:, dt, sc * 512:(sc + 1) * 512],
                            start=(dt == 0), stop=(dt == NDT - 1))
                    nc.vector.tensor_copy(
                        out=dst[fb][:, c2 * CW + sc * 512:
                                    c2 * CW + (sc + 1) * 512],
                        in_=ps[:])

                def v_pass(xt, c2, sb):
                    """V[s,d] for one s-block (cols 0-255 of a 1-bank tile)."""
                    ps = pp.tile([128, 512], F32, tag="pp", name="pv")
                    for dt in range(NDT):
                        nc.tensor.matmul(
                            ps[:, 0:256],
                            lhsT=xt[:, dt, sb * 128:(sb + 1) * 128],
                            rhs=w_sb["wv"][:, dt, :],
                            start=(dt == 0), stop=(dt == NDT - 1))
                    nc.vector.tensor_copy(
                        out=vx[:, :, c2 * 8 + sb, 1:HD + 1],
                        in_=ps[:, 0:256].rearrange("p (h d) -> p h d",
                                                   h=HPC))

                def attn_scores(h, q2, kb):
                    """scores + exp + region fixes -> es tile (SBUF bf16)."""
                    pr, hl = h // 2, h % 2
                    sps = spsp.tile([128, QW], F32, tag="s", name="s")
                    qsl = qt[pr][hl * 64:(hl + 1) * 64,
                                 q2 * QW:(q2 + 1) * QW]
                    ksl = kt[pr][hl * 64:(hl + 1) * 64,
                                 kb * 128:(kb + 1) * 128]
                    nc.tensor.matmul(sps[:], lhsT=ksl, rhs=qsl,
                                     start=True, stop=True)
                    if use_mask:
                        nc.vector.tensor_scalar_add(
                            sps[:], sps[:], mk[:, kb:kb + 1])
                    mi = _maj_side(kb, q2)
                    es = esp.tile([128, QW], BF16, tag="es", name="es")
                    nc.scalar.activation(
                        out=es[:], in_=sps[:],
                        func=mybir.ActivationFunctionType.Exp,
                        bias=cb[:, 0, mi, h:h + 1], scale=1.0)
                    # band fix + minority saturated side, both on DVE
                    a, b = _band_bounds(kb)
                    qlo = q2 * QW
                    bs, be = max(qlo, a), min(qlo + QW, b)
                    if bs < be:
                        w0 = bs - (kb - 1) * 128
                        nc.vector.tensor_mul(
                            es[:, bs - qlo:be - qlo],
                            es[:, bs - qlo:be - qlo],
                            eb_sb[:, mi, h, w0:w0 + (be - bs)])
                    if mi == 0:
                        ms, me = max(qlo, b), qlo + QW
                    else:
                        ms, me = qlo, min(qlo + QW, a)
                    if ms < me:
                        nc.vector.tensor_scalar_mul(
                            es[:, ms - qlo:me - qlo],
                            es[:, ms - qlo:me - qlo],
                            cb[:, 1, mi, h:h + 1])
                    return es

                def attn_ctx(ctx, h, kb, es):
                    nc.tensor.matmul(
                        ctx[:], lhsT=vx[:, h, kb, :], rhs=es[:],
                        start=(kb == 0), stop=(kb == NKB - 1))

                def attn_tail(ctx, h, q2):
                    # evacuate psum; row 0 = softmax denominators
                    # (normalization happens host-side during unsharding)
                    stg = stgp.tile([HD + 1, QW], F32, tag="stg", name="stg")
                    nc.vector.tensor_copy(out=stg[:], in_=ctx[:])
                    nc.gpsimd.dma_start(
                        out=outp[h, :, q2 * QW:(q2 + 1) * QW],
                        in_=stg[:])

                def pair_block(pr, q2, kbs, ctxs=None, held=None,
                               scores_only=False):
                    """One (head-pair, q chunk) span of k blocks.

                    The two heads' score matmuls are emitted back-to-back:
                    their contraction rows are disjoint 64-row groups, so
                    the PE runs them concurrently (row tiling).
                    """
                    h0, h1 = 2 * pr, 2 * pr + 1
                    for kb in kbs:
                        for i, h in enumerate((h0, h1)):
                            es = held.pop((h, kb), None) if held else None
                            if es is None and not scores_only:
                                es = attn_scores(h, q2, kb)
                            elif es is None:
                                held[(h, kb)] = attn_scores(h, q2, kb)
                                continue
                            attn_ctx(ctxs[i], h, kb, es)

                def pair_tail(ctxs, pr, q2):
                    attn_tail(ctxs[0], 2 * pr, q2)
                    attn_tail(ctxs[1], 2 * pr + 1, q2)

                def ctx_pair():
                    return [ctxp.tile([HD + 1, QW], F32, tag="ctx",
                                      name="ctx") for _ in range(2)]

                # ---- streaming + overlapped schedule ----
                # A-iters (scores+exp for a head pair, es held in SBUF) feed
                # ACT; B-iters (ctx accumulation from held es) and projection
                # passes are PE filler interleaved between A-iters so the PE
                # never waits in-queue on the exp chain.
                held = {}
                ctx_open = {}

                def A_iter(pr, q2, kb):
                    for h in (2 * pr, 2 * pr + 1):
                        held[(h, q2, kb)] = attn_scores(h, q2, kb)

                def B_iter(pr, q2, kb):
                    key = (pr, q2)
                    if key not in ctx_open:
                        ctx_open[key] = ctx_pair()
                    ctxs = ctx_open[key]
                    for i, h in enumerate((2 * pr, 2 * pr + 1)):
                        attn_ctx(ctxs[i], h, kb, held.pop((h, q2, kb)))

                def B_tail(pr, q2):
                    pair_tail(ctx_open.pop((pr, q2)), pr, q2)

                def ilv(a_ops, fill_ops):
                    """Interleave: distribute fills evenly among A-iters."""
                    n = len(a_ops)
                    out = []
                    fi = 0
                    for i, a in enumerate(a_ops):
                        out.append(a)
                        want = (i + 1) * len(fill_ops) // n
                        while fi < want:
                            out.append(fill_ops[fi])
                            fi += 1
                    out.extend(fill_ops[fi:])
                    return out

                def A(pr, q2, kbs):
                    return [lambda kb=kb: A_iter(pr, q2, kb) for kb in kbs]

                def Bb(pr, q2, kbs, tail=False):
                    ops = [lambda kb=kb: B_iter(pr, q2, kb) for kb in kbs]
                    if tail:
                        ops.append(lambda: B_tail(pr, q2))
                    return ops

                # input streams: xk/xq on the SP queue, xv on the gpsimd
                # queue (halves SP issue serialization)
                xk0 = stream_chunk(xk, "k", 0)
                xq0 = stream_chunk(xq, "q", 0)
                xv0 = stream_chunk(xv, "v", 0, q=nc.gpsimd)
                xk1 = stream_chunk(xk, "k", 1)
                xv1 = stream_chunk(xv, "v", 1, q=nc.gpsimd)
                xq1 = stream_chunk(xq, "q", 1)

                def KQ(w, dst, xt, c2, fb, sc):
                    return lambda: qk_pass(w, dst, xt, c2, fb, sc)

                def VP(xt, c2, sb):
                    return lambda: v_pass(xt, c2, sb)

                kp0 = [KQ("wk", kt, xk0, 0, fb, sc)
                       for fb in (0, 1) for sc in (0, 1)]
                qp0 = [KQ("wq", qt, xq0, 0, fb, sc)
                       for fb in (0, 1) for sc in (0, 1)]
                vp0 = [VP(xv0, 0, sb) for sb in range(8)]
                kp1 = [KQ("wk", kt, xk1, 1, fb, sc)
                       for fb in (0, 1) for sc in (0, 1)]
                vp1 = [VP(xv1, 1, sb) for sb in range(8)]
                qp1 = [KQ("wq", qt, xq1, 1, fb, sc)
                       for fb in (0, 1) for sc in (0, 1)]

                lo, hi = range(0, 8), range(8, NKB)
                sched = (
                    kp0 + qp0
                    + A(0, 0, lo)
                    + ilv(A(0, 1, lo), vp0)
                    + ilv(A(1, 0, lo), Bb(0, 0, lo))
                    + ilv(A(1, 1, lo), kp1)
                    + ilv(A(0, 0, hi), vp1[:5])
                    + ilv(A(0, 1, hi), vp1[5:] + Bb(0, 0, hi, tail=True))
                    + ilv(A(1, 0, hi), Bb(0, 1, lo))
                    + ilv(A(1, 1, hi), Bb(0, 1, hi, tail=True) + qp1)
                    + ilv(A(0, 2, range(NKB)),
                          Bb(1, 0, range(12)))
                    + ilv(A(1, 2, range(NKB)),
                          Bb(1, 0, range(12, NKB), tail=True)
                          + Bb(1, 1, range(11)))
                    + ilv(A(0, 3, range(NKB)),
                          Bb(1, 1, range(11, NKB), tail=True)
                          + Bb(0, 2, range(10)))
                    + ilv(A(1, 3, range(NKB)),
                          Bb(0, 2, range(10, NKB), tail=True)
                          + Bb(1, 2, range(9)))
                    + Bb(1, 2, range(9, NKB), tail=True)
                    + Bb(0, 3, range(NKB), tail=True)
                    + Bb(1, 3, range(NKB), tail=True)
                )
                for op in sched:
                    op()

    nc.finalize()
    return nc


_PROG_CACHE = {}


def _get_program(use_mask):
    key = bool(use_mask)
    if key not in _PROG_CACHE:
        _PROG_CACHE[key] = build_program(key)
    return _PROG_CACHE[key]


def kernel(query, key, value, key_mask, Wq, Wk, Wv, bias_table):
    import ml_dtypes
    bf16 = ml_dtypes.bfloat16
    f16 = np.float16

    query = np.asarray(query, dtype=np.float32)
    key = np.asarray(key, dtype=np.float32)
    value = np.asarray(value, dtype=np.float32)
    key_mask = np.asarray(key_mask, dtype=np.float32)
    Wq = np.asarray(Wq, dtype=np.float32)
    Wk = np.asarray(Wk, dtype=np.float32)
    Wv = np.asarray(Wv, dtype=np.float32)
    bias_table = np.asarray(bias_table, dtype=np.float32)

    use_mask = not np.all(key_mask == 1.0)
    nc = _get_program(use_mask)

    buckets = _rel_buckets()  # [2S-1] for rel = k-q in [-(S-1), S-1]
    g = bias_table[buckets]   # [2S-1, H] bias as function of rel
    in_maps = []
    for core in range(NCORES):
        b, hg = core // 4, core % 4
        hsl = slice(hg * HPC * HD, (hg + 1) * HPC * HD)
        heads = np.arange(hg * HPC, (hg + 1) * HPC)
        c31 = bias_table[31, heads]  # rel >= +128
        c15 = bias_table[15, heads]  # rel <= -128
        cmaj = np.stack([c31, c15])               # [side, h]
        cmin = np.stack([c15, c31])
        # -32 keeps the unnormalized exps in a sane fp32 range (softmax is
        # shift-invariant; numerator and denominator scale together)
        cv = np.stack([cmaj - 32.0, np.exp(cmin - cmaj)]).astype(np.float32)
        # band tables: ebt[side, h, p, w] = exp(g_h(p - w + 128) - cmaj)
        p = np.arange(128)[:, None]
        w = np.arange(EBW)[None, :]
        rel = p - w + 128                          # in (-256, 256)
        gh = g[rel + (S - 1)][:, :, heads]         # [128, EBW, HPC]
        ebt_np = np.empty((2, HPC, 128, EBW), np.float32)
        for mi in range(2):
            ebt_np[mi] = np.exp(
                gh - cmaj[mi][None, None, :]).transpose(2, 0, 1)
        im = {
            "xv": np.ascontiguousarray(value[b].T).astype(f16),
            "xq": np.ascontiguousarray(query[b].T).astype(f16),
            "xk": np.ascontiguousarray(key[b].T).astype(f16),
            "wq": np.ascontiguousarray(Wq[:, hsl]).astype(f16),
            "wk": np.ascontiguousarray(Wk[:, hsl]).astype(f16),
            "wv": np.ascontiguousarray(Wv[:, hsl]).astype(f16),
            "ebt": ebt_np.astype(bf16),
            "cvals": np.broadcast_to(cv, (128,) + cv.shape).copy(),
            "vones": np.ones((128, HPC * NKB), bf16),
        }
        if use_mask:
            madd = (-1e4 * (1.0 - key_mask[b])).astype(np.float32)
            im["mvals"] = np.ascontiguousarray(madd.reshape(NKB, 128).T)
        in_maps.append(im)

    res = run_bass_kernel_spmd(nc, in_maps, core_ids=list(range(NCORES)))
    out = np.empty((B, S, H * HD), np.float32)
    for core in range(NCORES):
        b, hg = core // 4, core % 4
        o = res.results[core]["out"]  # [HPC, HD+1, S]; row 0 = denominators
        for h in range(HPC):
            out[b, :, (hg * HPC + h) * HD:(hg * HPC + h + 1) * HD] = \
                (o[h, 1:] / o[h, 0:1]).T
    return out


# revision 19
# speedup vs baseline: 1.0043x; 1.0043x over previous
"""T5-style multi-head attention on 8 Trainium2 NeuronCores.

Problem: B=2, S=2048, D=1024, H=16 heads of 64; T5 relative-position bias
(32 buckets, max_distance=128), key mask, softmax, context.

Sharding: data-parallel over B (2) x tensor-parallel over head-groups of 4
(4 groups) = 8 cores.  Each core computes Q/K/V projections for its batch
and its 4 heads, then full attention for those heads.

v2: bf16 inputs/weights/activations (halves the input-DMA volume and
enables fast-weight-load), column-streamed projections software-pipelined
with the attention phase so the ACT engine (exp is the per-core compute
floor at ~126us) starts ~20us in instead of after all projections.

Device algorithm (per core), matmul cycles at 1 col/cycle in bf16:
  stream order: xk/xq/xv column-chunks of 1024; projections consume each
  chunk as it lands (Q/K as (X W)^T in [f, s] layout, V in [s, d] layout
  with a ones column for the softmax denominator).
  attention per (head, q2 chunk of 1024, k block of 128):
    scoresT[k,q] = K^T.T Q^T   (contraction d=64)
    expS = exp(scoresT + c_maj - 32) on ACT (c_maj = saturated-bucket bias)
    band fix (DVE) + minority saturated side (GPSIMD) as multiplicative
    corrections; ctxT[d|1,q] += V_ext.T expS accumulated over k blocks;
    row 0 of ctxT = softmax denominators (ones column rides along free).
  tail: reciprocal (DVE), partition broadcast (GPSIMD), scale, DMA out.

The first head's first-half k blocks are emitted before the second half
of the K/V streams arrive so ACT has work during the stream tail; its es
tiles for head h1 are held in SBUF until V lands.
"""

import numpy as np

import concourse.bacc as bacc
import concourse.tile as tile
from concourse import mybir
from concourse.bass_utils import run_bass_kernel_spmd

# problem dims (hardcoded per contract)
B = 2
S = 2048
DM = 1024
H = 16
HD = 64
NB = 32
MAXD = 128

HPC = 4          # heads per core
NCORES = 8
NDT = DM // 128  # 8 contraction tiles
NKB = S // 128   # 16 k blocks
NQ2 = 4          # q chunks of 512
QW = 512         # q chunk width
CW = 1024        # input stream chunk width
EBW = 384        # band table width

F32 = mybir.dt.float32
F32R = mybir.dt.float32r
BF16 = mybir.dt.bfloat16
F16 = mybir.dt.float16


def _rel_buckets():
    """T5 bidirectional bucket for rel = k - q in [-(S-1), S-1], fp32 math."""
    rel = np.arange(-(S - 1), S, dtype=np.int64)
    nb = NB // 2
    ret = (rel > 0).astype(np.int64) * nb
    rp = np.abs(rel)
    max_exact = nb // 2
    is_small = rp < max_exact
    rp_f = np.maximum(rp, 1).astype(np.float32)
    val = np.log(rp_f / np.float32(max_exact)) / np.float32(
        np.log(MAXD / max_exact)
    ) * np.float32(nb - max_exact)
    # XLA CPU f32->s32 convert rounds to nearest (cvtps2dq), not truncates
    val_large = max_exact + np.rint(val).astype(np.int32)
    val_large = np.minimum(val_large, nb - 1)
    return (ret + np.where(is_small, rp, val_large)).astype(np.int64)  # [2S-1]


def _band_bounds(kb):
    """Columns [a,b) of the non-saturated diagonal band for k block kb."""
    a = max(0, (kb - 1) * 128)
    b = min(S, (kb + 2) * 128)
    return a, b


def _maj_side(kb, q2):
    """Majority saturated side for (k block, q chunk): 0 -> bucket31 (q<a),
    1 -> bucket15 (q>=b)."""
    qlo, qhi = q2 * QW, (q2 + 1) * QW
    a, b = _band_bounds(kb)
    len31 = max(0, min(qhi, a) - qlo)
    len15 = max(0, qhi - max(qlo, b))
    return 0 if len31 >= len15 else 1


def build_program(use_mask, reps=1):
    nc = bacc.Bacc("TRN2", target_bir_lowering=False, debug=False,
                   num_devices=NCORES)

    xv = nc.dram_tensor("xv", [DM, S], F16, kind="ExternalInput").ap()
    xq = nc.dram_tensor("xq", [DM, S], F16, kind="ExternalInput").ap()
    xk = nc.dram_tensor("xk", [DM, S], F16, kind="ExternalInput").ap()
    wq = nc.dram_tensor("wq", [DM, HPC * HD], F16, kind="ExternalInput").ap()
    wk = nc.dram_tensor("wk", [DM, HPC * HD], F16, kind="ExternalInput").ap()
    wv = nc.dram_tensor("wv", [DM, HPC * HD], F16, kind="ExternalInput").ap()
    # band tables exp(g_h(rel) - c_maj): [side, head, 128, EBW]
    ebt = nc.dram_tensor("ebt", [2, HPC, 128, EBW], BF16,
                         kind="ExternalInput").ap()
    # per-(side, head): exp bias constant c_maj and minority ratio
    # cvals[0, side, h] = c_maj - 32 ; cvals[1, side, h] = exp(c_min - c_maj)
    cvals = nc.dram_tensor("cvals", [128, 2, 2, HPC], F32,
                           kind="ExternalInput").ap()
    vones = nc.dram_tensor("vones", [128, HPC * NKB], BF16,
                           kind="ExternalInput").ap()
    if use_mask:
        # additive mask term -1e4*(1-mask) laid out [128, NKB]
        mvals = nc.dram_tensor("mvals", [128, NKB], F32,
                               kind="ExternalInput").ap()
    # row 0 = softmax denominators, rows 1..64 = unnormalized context;
    # the division happens host-side during unsharding
    outp = nc.dram_tensor("out", [HPC, HD + 1, S], F32,
                          kind="ExternalOutput").ap()

    with tile.TileContext(nc) as tc:
        with tc.tile_pool(name="const", bufs=1) as const, \
             tc.tile_pool(name="qkt", bufs=1) as qkt, \
             tc.tile_pool(name="xs", bufs=3) as xs, \
             tc.tile_pool(name="esp", bufs=80) as esp, \
             tc.tile_pool(name="stgp", bufs=3) as stgp:

            # ---- resident constants ----
            w_sb = {}
            for nm, src in (("wk", wk), ("wq", wq), ("wv", wv)):
                t = const.tile([128, NDT, HPC * HD], F16, tag=nm, name=nm)
                nc.gpsimd.dma_start(
                    out=t[:], in_=src.rearrange("(dt p) f -> p dt f", p=128))
                w_sb[nm] = t
            cb = const.tile([128, 2, 2, HPC], F32, tag="cb", name="cb")
            nc.gpsimd.dma_start(out=cb[:], in_=cvals[:])
            eb_sb = const.tile([128, 2, HPC, EBW], BF16, tag="eb", name="eb")
            nc.gpsimd.dma_start(out=eb_sb[:],
                                in_=ebt.rearrange("m h p w -> p m h w"))
            if use_mask:
                mk = const.tile([128, NKB], F32, tag="mk", name="mk")
                nc.gpsimd.dma_start(out=mk[:], in_=mvals[:])

            # Q^T/K^T per pair: [128(2 heads x 64d), S] bf16
            qt = [qkt.tile([128, S], F16, tag=f"qt{p}", name=f"qt{p}")
                  for p in range(2)]
            kt = [qkt.tile([128, S], F16, tag=f"kt{p}", name=f"kt{p}")
                  for p in range(2)]
            # V_ext: [128(k in block), head, kblock, 65(1|d)]
            vx = qkt.tile([128, HPC, NKB, HD + 1], BF16, tag="vx", name="vx")
            nc.gpsimd.dma_start(
                out=vx[:, :, :, 0:1],
                in_=vones.rearrange("p (h k one) -> p h k one", h=HPC, one=1))

            for _rep in range(reps):
              # psum pools ([*,512] f32 = 1 bank each):
              # sps 4 + ctx 2 + pp 2 = 8 banks
              with tc.tile_pool(name="pp", bufs=2, space="PSUM") as pp, \
                   tc.tile_pool(name="spsp", bufs=4, space="PSUM") as spsp, \
                   tc.tile_pool(name="ctxp", bufs=2, space="PSUM") as ctxp:

                def stream_chunk(src, tag, c2, q=None):
                    """One [dm, 1024]-column chunk as 4 DMAs of dt-pairs
                    (few issues, 2KB descriptors, fine-grained deps)."""
                    q = q or nc.sync
                    t = xs.tile([128, NDT, CW], F16, tag="x", name=f"x{tag}")
                    for dp in range(4):
                        q.dma_start(
                            out=t[:, 2 * dp:2 * dp + 2, :],
                            in_=src[256 * dp:256 * (dp + 1),
                                    c2 * CW:(c2 + 1) * CW].rearrange(
                                "(dt p) w -> p dt w", p=128))
                    return t

                def qk_pass(wname, dst, xt, c2, fb, sc):
                    """One 512-col psum pass of (X W)^T."""
                    ps = pp.tile([128, 512], F32, tag="pp", name="pj")
                    for dt in range(NDT):
                        nc.tensor.matmul(
                            ps[:],
                            lhsT=w_sb[wname][:, dt,
                                             fb * 128:(fb + 1) * 128],
                            rhs=xt[:, dt, sc * 512:(sc + 1) * 512],
                            start=(dt == 0), stop=(dt == NDT - 1))
                    nc.vector.tensor_copy(
                        out=dst[fb][:, c2 * CW + sc * 512:
                                    c2 * CW + (sc + 1) * 512],
                        in_=ps[:])

                def v_pass(xt, c2, sb):
                    """V[s,d] for one s-block (cols 0-255 of a 1-bank tile)."""
                    ps = pp.tile([128, 512], F32, tag="pp", name="pv")
                    for dt in range(NDT):
                        nc.tensor.matmul(
                            ps[:, 0:256],
                            lhsT=xt[:, dt, sb * 128:(sb + 1) * 128],
                            rhs=w_sb["wv"][:, dt, :],
                            start=(dt == 0), stop=(dt == NDT - 1))
                    nc.vector.tensor_copy(
                        out=vx[:, :, c2 * 8 + sb, 1:HD + 1],
                        in_=ps[:, 0:256].rearrange("p (h d) -> p h d",
                                                   h=HPC))

                def attn_scores(h, q2, kb):
                    """scores + exp + region fixes -> es tile (SBUF bf16)."""
                    pr, hl = h // 2, h % 2
                    sps = spsp.tile([128, QW], F32, tag="s", name="s")
                    qsl = qt[pr][hl * 64:(hl + 1) * 64,
                                 q2 * QW:(q2 + 1) * QW]
                    ksl = kt[pr][hl * 64:(hl + 1) * 64,
                                 kb * 128:(kb + 1) * 128]
                    nc.tensor.matmul(sps[:], lhsT=ksl, rhs=qsl,
                                     start=True, stop=True)
                    if use_mask:
                        nc.vector.tensor_scalar_add(
                            sps[:], sps[:], mk[:, kb:kb + 1])
                    mi = _maj_side(kb, q2)
                    es = esp.tile([128, QW], BF16, tag="es", name="es")
                    nc.scalar.activation(
                        out=es[:], in_=sps[:],
                        func=mybir.ActivationFunctionType.Exp,
                        bias=cb[:, 0, mi, h:h + 1], scale=1.0)
                    # band fix + minority saturated side, both on DVE
                    a, b = _band_bounds(kb)
                    qlo = q2 * QW
                    bs, be = max(qlo, a), min(qlo + QW, b)
                    if bs < be:
                        w0 = bs - (kb - 1) * 128
                        nc.vector.tensor_mul(
                            es[:, bs - qlo:be - qlo],
                            es[:, bs - qlo:be - qlo],
                            eb_sb[:, mi, h, w0:w0 + (be - bs)])
                    if mi == 0:
                        ms, me = max(qlo, b), qlo + QW
                    else:
                        ms, me = qlo, min(qlo + QW, a)
                    if ms < me:
                        nc.vector.tensor_scalar_mul(
                            es[:, ms - qlo:me - qlo],
                            es[:, ms - qlo:me - qlo],
                            cb[:, 1, mi, h:h + 1])
                    return es

                def attn_ctx(ctx, h, kb, es):
                    nc.tensor.matmul(
                        ctx[:], lhsT=vx[:, h, kb, :], rhs=es[:],
                        start=(kb == 0), stop=(kb == NKB - 1))

                def attn_tail(ctx, h, q2):
                    # evacuate psum; row 0 = softmax denominators
                    # (normalization happens host-side during unsharding)
                    stg = stgp.tile([HD + 1, QW], F32, tag="stg", name="stg")
                    nc.vector.tensor_copy(out=stg[:], in_=ctx[:])
                    nc.gpsimd.dma_start(
                        out=outp[h, :, q2 * QW:(q2 + 1) * QW],
                        in_=stg[:])

                def pair_block(pr, q2, kbs, ctxs=None, held=None,
                               scores_only=False):
                    """One (head-pair, q chunk) span of k blocks.

                    The two heads' score matmuls are emitted back-to-back:
                    their contraction rows are disjoint 64-row groups, so
                    the PE runs them concurrently (row tiling).
                    """
                    h0, h1 = 2 * pr, 2 * pr + 1
                    for kb in kbs:
                        for i, h in enumerate((h0, h1)):
                            es = held.pop((h, kb), None) if held else None
                            if es is None and not scores_only:
                                es = attn_scores(h, q2, kb)
                            elif es is None:
                                held[(h, kb)] = attn_scores(h, q2, kb)
                                continue
                            attn_ctx(ctxs[i], h, kb, es)

                def pair_tail(ctxs, pr, q2):
                    attn_tail(ctxs[0], 2 * pr, q2)
                    attn_tail(ctxs[1], 2 * pr + 1, q2)

                def ctx_pair():
                    return [ctxp.tile([HD + 1, QW], F32, tag="ctx",
                                      name="ctx") for _ in range(2)]

                # ---- streaming + overlapped schedule ----
                # A-ops (scores+exp, es held in SBUF) gate ACT and are laid
                # out as a ladder behind the input stream; projection passes
                # are embedded at their data-arrival points.  B-ops (ctx
                # accumulation + tails) drain from a FIFO in psum-pool order,
                # dependency-gated and spread evenly so the PE never idles
                # or blocks in-queue on the exp chain.
                held = {}
                ctx_open = {}

                def A_iter(pr, q2, kb):
                    for h in (2 * pr, 2 * pr + 1):
                        held[(h, q2, kb)] = attn_scores(h, q2, kb)

                def B_iter(pr, q2, kb):
                    key = (pr, q2)
                    if key not in ctx_open:
                        ctx_open[key] = ctx_pair()
                    ctxs = ctx_open[key]
                    for i, h in enumerate((2 * pr, 2 * pr + 1)):
                        attn_ctx(ctxs[i], h, kb, held.pop((h, q2, kb)))

                def B_tail(pr, q2):
                    pair_tail(ctx_open.pop((pr, q2)), pr, q2)

                xk0 = stream_chunk(xk, "k", 0)
                xq0 = stream_chunk(xq, "q", 0)
                xv0 = stream_chunk(xv, "v", 0, q=nc.gpsimd)
                xk1 = stream_chunk(xk, "k", 1)
                xv1 = stream_chunk(xv, "v", 1, q=nc.gpsimd)
                xq1 = stream_chunk(xq, "q", 1)

                def KQ(w, dst, xt, c2, fb, sc):
                    return ("P", lambda: qk_pass(w, dst, xt, c2, fb, sc))

                def VP(xt, c2, sb):
                    return ("P", lambda: v_pass(xt, c2, sb))

                def Ao(pr, q2, kbs):
                    return [("A", pr, q2, kb) for kb in kbs]

                vp0 = [VP(xv0, 0, sb) for sb in range(8)]
                vp1 = [VP(xv1, 1, sb) for sb in range(8)]

                # A-list ladder: each (pr, q2, kb) A-op behind the proj
                # passes that produce its kt/qt slices
                alist = []
                alist += [KQ("wk", kt, xk0, 0, 0, 0), KQ("wq", qt, xq0, 0, 0, 0)]
                alist += Ao(0, 0, range(0, 4))
                alist += [KQ("wk", kt, xk0, 0, 0, 1)] + Ao(0, 0, range(4, 8))
                alist += [KQ("wq", qt, xq0, 0, 0, 1)] + Ao(0, 1, range(0, 4))
                alist += [KQ("wk", kt, xk0, 0, 1, 0), KQ("wq", qt, xq0, 0, 1, 0)]
                for i, a in enumerate(Ao(0, 1, range(4, 8)) + Ao(1, 0, range(0, 4))):
                    alist += [a, vp0[i]]
                alist += [KQ("wk", kt, xk0, 0, 1, 1), KQ("wq", qt, xq0, 0, 1, 1)]
                alist += Ao(1, 0, range(4, 8)) + Ao(1, 1, range(0, 8))
                alist += [KQ("wk", kt, xk1, 1, 0, 0), KQ("wk", kt, xk1, 1, 0, 1)]
                alist += Ao(0, 0, range(8, 12)) + Ao(0, 0, range(12, 16))
                for i, a in enumerate(Ao(0, 1, range(8, 16))):
                    alist += [a, vp1[i]]
                alist += [KQ("wk", kt, xk1, 1, 1, 0), KQ("wk", kt, xk1, 1, 1, 1)]
                alist += Ao(1, 0, range(8, 16))
                alist += Ao(1, 1, range(8, 16))
                alist += [KQ("wq", qt, xq1, 1, 0, 0)] + Ao(0, 2, range(NKB))
                alist += [KQ("wq", qt, xq1, 1, 1, 0)] + Ao(1, 2, range(NKB))
                alist += [KQ("wq", qt, xq1, 1, 0, 1)] + Ao(0, 3, range(NKB))
                alist += [KQ("wq", qt, xq1, 1, 1, 1)] + Ao(1, 3, range(NKB))

                # positions for dependency gating
                apos = {}
                vp_last = {0: 0, 1: 0}
                for i, op in enumerate(alist):
                    if op[0] == "A":
                        apos[op[1:]] = i
                for i, op in enumerate(alist):
                    if op[0] == "P" and i < len(alist):
                        pass
                # vx halves: find index after last vp0/vp1 in alist
                idx = 0
                for i, op in enumerate(alist):
                    if op is vp0[-1]:
                        vp_last[0] = i
                    if op is vp1[-1]:
                        vp_last[1] = i

                # B-FIFO in psum-pool order: (min_a_index, op)
                border = [(0, 0), (0, 1), (1, 0), (1, 1),
                          (0, 2), (1, 2), (0, 3), (1, 3)]
                bfifo = []
                for pr, q2 in border:
                    for kb in range(NKB):
                        gate = apos[(pr, q2, kb)] + 3
                        gate = max(gate, vp_last[0 if kb < 8 else 1] + 1)
                        bfifo.append((gate, lambda pr=pr, q2=q2, kb=kb:
                                      B_iter(pr, q2, kb)))
                    bfifo.append((apos[(pr, q2, NKB - 1)] + 3,
                                  lambda pr=pr, q2=q2: B_tail(pr, q2)))

                # emit: walk the A-list, draining the B-FIFO proportionally
                total = len(alist)
                nb = len(bfifo)
                bi = 0
                for i, op in enumerate(alist):
                    if op[0] == "A":
                        A_iter(op[1], op[2], op[3])
                    else:
                        op[1]()
                    want = nb * (i + 1) // total + 2
                    while bi < min(want, nb) and bfifo[bi][0] <= i:
                        bfifo[bi][1]()
                        bi += 1
                while bi < nb:
                    bfifo[bi][1]()
                    bi += 1

    nc.finalize()
    return nc


_PROG_CACHE = {}


def _get_program(use_mask):
    key = bool(use_mask)
    if key not in _PROG_CACHE:
        _PROG_CACHE[key] = build_program(key)
    return _PROG_CACHE[key]


def kernel(query, key, value, key_mask, Wq, Wk, Wv, bias_table):
    import ml_dtypes
    bf16 = ml_dtypes.bfloat16
    f16 = np.float16

    query = np.asarray(query, dtype=np.float32)
    key = np.asarray(key, dtype=np.float32)
    value = np.asarray(value, dtype=np.float32)
    key_mask = np.asarray(key_mask, dtype=np.float32)
    Wq = np.asarray(Wq, dtype=np.float32)
    Wk = np.asarray(Wk, dtype=np.float32)
    Wv = np.asarray(Wv, dtype=np.float32)
    bias_table = np.asarray(bias_table, dtype=np.float32)

    use_mask = not np.all(key_mask == 1.0)
    nc = _get_program(use_mask)

    buckets = _rel_buckets()  # [2S-1] for rel = k-q in [-(S-1), S-1]
    g = bias_table[buckets]   # [2S-1, H] bias as function of rel
    in_maps = []
    for core in range(NCORES):
        b, hg = core // 4, core % 4
        hsl = slice(hg * HPC * HD, (hg + 1) * HPC * HD)
        heads = np.arange(hg * HPC, (hg + 1) * HPC)
        c31 = bias_table[31, heads]  # rel >= +128
        c15 = bias_table[15, heads]  # rel <= -128
        cmaj = np.stack([c31, c15])               # [side, h]
        cmin = np.stack([c15, c31])
        # -32 keeps the unnormalized exps in a sane fp32 range (softmax is
        # shift-invariant; numerator and denominator scale together)
        cv = np.stack([cmaj - 32.0, np.exp(cmin - cmaj)]).astype(np.float32)
        # band tables: ebt[side, h, p, w] = exp(g_h(p - w + 128) - cmaj)
        p = np.arange(128)[:, None]
        w = np.arange(EBW)[None, :]
        rel = p - w + 128                          # in (-256, 256)
        gh = g[rel + (S - 1)][:, :, heads]         # [128, EBW, HPC]
        ebt_np = np.empty((2, HPC, 128, EBW), np.float32)
        for mi in range(2):
            ebt_np[mi] = np.exp(
                gh - cmaj[mi][None, None, :]).transpose(2, 0, 1)
        im = {
            "xv": np.ascontiguousarray(value[b].T).astype(f16),
            "xq": np.ascontiguousarray(query[b].T).astype(f16),
            "xk": np.ascontiguousarray(key[b].T).astype(f16),
            "wq": np.ascontiguousarray(Wq[:, hsl]).astype(f16),
            "wk": np.ascontiguousarray(Wk[:, hsl]).astype(f16),
            "wv": np.ascontiguousarray(Wv[:, hsl]).astype(f16),
            "ebt": ebt_np.astype(bf16),
            "cvals": np.broadcast_to(cv, (128,) + cv.shape).copy(),
            "vones": np.ones((128, HPC * NKB), bf16),
        }
        if use_mask:
            madd = (-1e4 * (1.0 - key_mask[b])).astype(np.float32)
            im["mvals"] = np.ascontiguousarray(madd.reshape(NKB, 128).T)
        in_maps.append(im)

    res = run_bass_kernel_spmd(nc, in_maps, core_ids=list(range(NCORES)))
    out = np.empty((B, S, H * HD), np.float32)
    for core in range(NCORES):
        b, hg = core // 4, core % 4
        o = res.results[core]["out"]  # [HPC, HD+1, S]; row 0 = denominators
        for h in range(HPC):
            out[b, :, (hg * HPC + h) * HD:(hg * HPC + h + 1) * HD] = \
                (o[h, 1:] / o[h, 0:1]).T
    return out


# revision 21
# speedup vs baseline: 1.0383x; 1.0339x over previous
"""T5-style multi-head attention on 8 Trainium2 NeuronCores.

Problem: B=2, S=2048, D=1024, H=16 heads of 64; T5 relative-position bias
(32 buckets, max_distance=128), key mask, softmax, context.

Sharding: data-parallel over B (2) x tensor-parallel over head-groups of 4
(4 groups) = 8 cores.  Each core computes Q/K/V projections for its batch
and its 4 heads, then full attention for those heads.

v2: bf16 inputs/weights/activations (halves the input-DMA volume and
enables fast-weight-load), column-streamed projections software-pipelined
with the attention phase so the ACT engine (exp is the per-core compute
floor at ~126us) starts ~20us in instead of after all projections.

Device algorithm (per core), matmul cycles at 1 col/cycle in bf16:
  stream order: xk/xq/xv column-chunks of 1024; projections consume each
  chunk as it lands (Q/K as (X W)^T in [f, s] layout, V in [s, d] layout
  with a ones column for the softmax denominator).
  attention per (head, q2 chunk of 1024, k block of 128):
    scoresT[k,q] = K^T.T Q^T   (contraction d=64)
    expS = exp(scoresT + c_maj - 32) on ACT (c_maj = saturated-bucket bias)
    band fix (DVE) + minority saturated side (GPSIMD) as multiplicative
    corrections; ctxT[d|1,q] += V_ext.T expS accumulated over k blocks;
    row 0 of ctxT = softmax denominators (ones column rides along free).
  tail: reciprocal (DVE), partition broadcast (GPSIMD), scale, DMA out.

The first head's first-half k blocks are emitted before the second half
of the K/V streams arrive so ACT has work during the stream tail; its es
tiles for head h1 are held in SBUF until V lands.
"""

import numpy as np

import concourse.bacc as bacc
import concourse.tile as tile
from concourse import mybir
from concourse.bass_utils import run_bass_kernel_spmd

# problem dims (hardcoded per contract)
B = 2
S = 2048
DM = 1024
H = 16
HD = 64
NB = 32
MAXD = 128

HPC = 4          # heads per core
NCORES = 8
NDT = DM // 128  # 8 contraction tiles
NKB = S // 128   # 16 k blocks
NQ2 = 4          # q chunks of 512
QW = 512         # q chunk width
CW = 1024        # input stream chunk width
EBW = 384        # band table width

F32 = mybir.dt.float32
F32R = mybir.dt.float32r
BF16 = mybir.dt.bfloat16
F16 = mybir.dt.float16


def _rel_buckets():
    """T5 bidirectional bucket for rel = k - q in [-(S-1), S-1], fp32 math."""
    rel = np.arange(-(S - 1), S, dtype=np.int64)
    nb = NB // 2
    ret = (rel > 0).astype(np.int64) * nb
    rp = np.abs(rel)
    max_exact = nb // 2
    is_small = rp < max_exact
    rp_f = np.maximum(rp, 1).astype(np.float32)
    val = np.log(rp_f / np.float32(max_exact)) / np.float32(
        np.log(MAXD / max_exact)
    ) * np.float32(nb - max_exact)
    # XLA CPU f32->s32 convert rounds to nearest (cvtps2dq), not truncates
    val_large = max_exact + np.rint(val).astype(np.int32)
    val_large = np.minimum(val_large, nb - 1)
    return (ret + np.where(is_small, rp, val_large)).astype(np.int64)  # [2S-1]


def _band_bounds(kb):
    """Columns [a,b) of the non-saturated diagonal band for k block kb."""
    a = max(0, (kb - 1) * 128)
    b = min(S, (kb + 2) * 128)
    return a, b


def _maj_side(kb, q2):
    """Majority saturated side for (k block, q chunk): 0 -> bucket31 (q<a),
    1 -> bucket15 (q>=b)."""
    qlo, qhi = q2 * QW, (q2 + 1) * QW
    a, b = _band_bounds(kb)
    len31 = max(0, min(qhi, a) - qlo)
    len15 = max(0, qhi - max(qlo, b))
    return 0 if len31 >= len15 else 1


def build_program(use_mask, reps=1):
    nc = bacc.Bacc("TRN2", target_bir_lowering=False, debug=False,
                   num_devices=NCORES)

    xv = nc.dram_tensor("xv", [DM, S], F16, kind="ExternalInput").ap()
    xq = nc.dram_tensor("xq", [DM, S], F16, kind="ExternalInput").ap()
    xk = nc.dram_tensor("xk", [DM, S], F16, kind="ExternalInput").ap()
    # weights pre-arranged host-side to [128, dt, f] so the load is one
    # contiguous-descriptor DMA (gathers here would steal DMA engines from
    # the startup-critical x stream)
    wq = nc.dram_tensor("wq", [128, NDT * HPC * HD], F16,
                        kind="ExternalInput").ap()
    wk = nc.dram_tensor("wk", [128, NDT * HPC * HD], F16,
                        kind="ExternalInput").ap()
    wv = nc.dram_tensor("wv", [128, NDT * HPC * HD], F16,
                        kind="ExternalInput").ap()
    # band tables exp(g_h(rel) - c_maj), pre-arranged to partition-major
    ebt = nc.dram_tensor("ebt", [128, 2 * HPC * EBW], BF16,
                         kind="ExternalInput").ap()
    # per-(side, head): exp bias constant c_maj and minority ratio
    # cvals[0, side, h] = c_maj - 32 ; cvals[1, side, h] = exp(c_min - c_maj)
    cvals = nc.dram_tensor("cvals", [128, 2, 2, HPC], F32,
                           kind="ExternalInput").ap()
    if use_mask:
        # additive mask term -1e4*(1-mask) laid out [128, NKB]
        mvals = nc.dram_tensor("mvals", [128, NKB], F32,
                               kind="ExternalInput").ap()
    # row 0 = softmax denominators, rows 1..64 = unnormalized context;
    # the division happens host-side during unsharding
    outp = nc.dram_tensor("out", [HPC, HD + 1, S], F32,
                          kind="ExternalOutput").ap()

    with tile.TileContext(nc) as tc:
        with tc.tile_pool(name="const", bufs=1) as const, \
             tc.tile_pool(name="qkt", bufs=1) as qkt, \
             tc.tile_pool(name="xs", bufs=3) as xs, \
             tc.tile_pool(name="esp", bufs=80) as esp, \
             tc.tile_pool(name="stgp", bufs=3) as stgp:

            # ---- resident constants ----
            w_sb = {}
            for nm, src in (("wk", wk), ("wq", wq), ("wv", wv)):
                t = const.tile([128, NDT, HPC * HD], F16, tag=nm, name=nm)
                nc.gpsimd.dma_start(
                    out=t[:], in_=src.rearrange("p (dt f) -> p dt f", dt=NDT))
                w_sb[nm] = t
            cb = const.tile([128, 2, 2, HPC], F32, tag="cb", name="cb")
            nc.gpsimd.dma_start(out=cb[:], in_=cvals[:])
            eb_sb = const.tile([128, 2, HPC, EBW], BF16, tag="eb", name="eb")
            nc.gpsimd.dma_start(
                out=eb_sb[:],
                in_=ebt.rearrange("p (m h w) -> p m h w", m=2, h=HPC))
            if use_mask:
                mk = const.tile([128, NKB], F32, tag="mk", name="mk")
                nc.gpsimd.dma_start(out=mk[:], in_=mvals[:])

            # Q^T/K^T per pair: [128(2 heads x 64d), S] bf16
            qt = [qkt.tile([128, S], F16, tag=f"qt{p}", name=f"qt{p}")
                  for p in range(2)]
            kt = [qkt.tile([128, S], F16, tag=f"kt{p}", name=f"kt{p}")
                  for p in range(2)]
            # V_ext: [128(k in block), head, kblock, 65(1|d)]
            vx = qkt.tile([128, HPC, NKB, HD + 1], BF16, tag="vx", name="vx")
            nc.vector.memset(vx[:], 1.0)

            for _rep in range(reps):
              # psum pools ([*,512] f32 = 1 bank each):
              # sps 4 + ctx 2 + pp 2 = 8 banks
              with tc.tile_pool(name="pp", bufs=2, space="PSUM") as pp, \
                   tc.tile_pool(name="spsp", bufs=4, space="PSUM") as spsp, \
                   tc.tile_pool(name="ctxp", bufs=2, space="PSUM") as ctxp:

                def stream_chunk(src, tag, c2, q=None):
                    """One [dm, 1024]-column chunk as 4 DMAs of dt-pairs
                    (few issues, 2KB descriptors, fine-grained deps)."""
                    q = q or nc.sync
                    t = xs.tile([128, NDT, CW], F16, tag="x", name=f"x{tag}")
                    for dp in range(4):
                        q.dma_start(
                            out=t[:, 2 * dp:2 * dp + 2, :],
                            in_=src[256 * dp:256 * (dp + 1),
                                    c2 * CW:(c2 + 1) * CW].rearrange(
                                "(dt p) w -> p dt w", p=128))
                    return t

                def qk_pass(wname, dst, xt, c2, fb, sc):
                    """One 512-col psum pass of (X W)^T."""
                    ps = pp.tile([128, 512], F32, tag="pp", name="pj")
                    for dt in range(NDT):
                        nc.tensor.matmul(
                            ps[:],
                            lhsT=w_sb[wname][:, dt,
                                             fb * 128:(fb + 1) * 128],
                            rhs=xt[:, dt, sc * 512:(sc + 1) * 512],
                            start=(dt == 0), stop=(dt == NDT - 1))
                    nc.vector.tensor_copy(
                        out=dst[fb][:, c2 * CW + sc * 512:
                                    c2 * CW + (sc + 1) * 512],
                        in_=ps[:])

                def v_pass(xt, c2, sb):
                    """V[s,d] for one s-block (cols 0-255 of a 1-bank tile)."""
                    ps = pp.tile([128, 512], F32, tag="pp", name="pv")
                    for dt in range(NDT):
                        nc.tensor.matmul(
                            ps[:, 0:256],
                            lhsT=xt[:, dt, sb * 128:(sb + 1) * 128],
                            rhs=w_sb["wv"][:, dt, :],
                            start=(dt == 0), stop=(dt == NDT - 1))
                    nc.vector.tensor_copy(
                        out=vx[:, :, c2 * 8 + sb, 1:HD + 1],
                        in_=ps[:, 0:256].rearrange("p (h d) -> p h d",
                                                   h=HPC))

                def attn_scores(h, q2, kb):
                    """scores + exp + region fixes -> es tile (SBUF bf16)."""
                    pr, hl = h // 2, h % 2
                    sps = spsp.tile([128, QW], F32, tag="s", name="s")
                    qsl = qt[pr][hl * 64:(hl + 1) * 64,
                                 q2 * QW:(q2 + 1) * QW]
                    ksl = kt[pr][hl * 64:(hl + 1) * 64,
                                 kb * 128:(kb + 1) * 128]
                    nc.tensor.matmul(sps[:], lhsT=ksl, rhs=qsl,
                                     start=True, stop=True)
                    if use_mask:
                        nc.vector.tensor_scalar_add(
                            sps[:], sps[:], mk[:, kb:kb + 1])
                    mi = _maj_side(kb, q2)
                    es = esp.tile([128, QW], BF16, tag="es", name="es")
                    nc.scalar.activation(
                        out=es[:], in_=sps[:],
                        func=mybir.ActivationFunctionType.Exp,
                        bias=cb[:, 0, mi, h:h + 1], scale=1.0)
                    # band fix + minority saturated side, both on DVE
                    a, b = _band_bounds(kb)
                    qlo = q2 * QW
                    bs, be = max(qlo, a), min(qlo + QW, b)
                    if bs < be:
                        w0 = bs - (kb - 1) * 128
                        nc.vector.tensor_mul(
                            es[:, bs - qlo:be - qlo],
                            es[:, bs - qlo:be - qlo],
                            eb_sb[:, mi, h, w0:w0 + (be - bs)])
                    if mi == 0:
                        ms, me = max(qlo, b), qlo + QW
                    else:
                        ms, me = qlo, min(qlo + QW, a)
                    if ms < me:
                        nc.vector.tensor_scalar_mul(
                            es[:, ms - qlo:me - qlo],
                            es[:, ms - qlo:me - qlo],
                            cb[:, 1, mi, h:h + 1])
                    return es

                def attn_ctx(ctx, h, kb, es):
                    nc.tensor.matmul(
                        ctx[:], lhsT=vx[:, h, kb, :], rhs=es[:],
                        start=(kb == 0), stop=(kb == NKB - 1))

                def attn_tail(ctx, h, q2):
                    # evacuate psum; row 0 = softmax denominators
                    # (normalization happens host-side during unsharding)
                    stg = stgp.tile([HD + 1, QW], F32, tag="stg", name="stg")
                    nc.vector.tensor_copy(out=stg[:], in_=ctx[:])
                    nc.gpsimd.dma_start(
                        out=outp[h, :, q2 * QW:(q2 + 1) * QW],
                        in_=stg[:])

                def pair_block(pr, q2, kbs, ctxs=None, held=None,
                               scores_only=False):
                    """One (head-pair, q chunk) span of k blocks.

                    The two heads' score matmuls are emitted back-to-back:
                    their contraction rows are disjoint 64-row groups, so
                    the PE runs them concurrently (row tiling).
                    """
                    h0, h1 = 2 * pr, 2 * pr + 1
                    for kb in kbs:
                        for i, h in enumerate((h0, h1)):
                            es = held.pop((h, kb), None) if held else None
                            if es is None and not scores_only:
                                es = attn_scores(h, q2, kb)
                            elif es is None:
                                held[(h, kb)] = attn_scores(h, q2, kb)
                                continue
                            attn_ctx(ctxs[i], h, kb, es)

                def pair_tail(ctxs, pr, q2):
                    attn_tail(ctxs[0], 2 * pr, q2)
                    attn_tail(ctxs[1], 2 * pr + 1, q2)

                def ctx_pair():
                    return [ctxp.tile([HD + 1, QW], F32, tag="ctx",
                                      name="ctx") for _ in range(2)]

                # ---- streaming + overlapped schedule ----
                # A-ops (scores+exp, es held in SBUF) gate ACT and are laid
                # out as a ladder behind the input stream; projection passes
                # are embedded at their data-arrival points.  B-ops (ctx
                # accumulation + tails) drain from a FIFO in psum-pool order,
                # dependency-gated and spread evenly so the PE never idles
                # or blocks in-queue on the exp chain.
                held = {}
                ctx_open = {}

                def A_iter(pr, q2, kb):
                    for h in (2 * pr, 2 * pr + 1):
                        held[(h, q2, kb)] = attn_scores(h, q2, kb)

                def B_iter(pr, q2, kb):
                    key = (pr, q2)
                    if key not in ctx_open:
                        ctx_open[key] = ctx_pair()
                    ctxs = ctx_open[key]
                    for i, h in enumerate((2 * pr, 2 * pr + 1)):
                        attn_ctx(ctxs[i], h, kb, held.pop((h, q2, kb)))

                def B_tail(pr, q2):
                    pair_tail(ctx_open.pop((pr, q2)), pr, q2)

                xk0 = stream_chunk(xk, "k", 0)
                xq0 = stream_chunk(xq, "q", 0)
                xv0 = stream_chunk(xv, "v", 0, q=nc.gpsimd)
                xk1 = stream_chunk(xk, "k", 1)
                xv1 = stream_chunk(xv, "v", 1, q=nc.gpsimd)
                xq1 = stream_chunk(xq, "q", 1)

                def KQ(w, dst, xt, c2, fb, sc):
                    return ("P", lambda: qk_pass(w, dst, xt, c2, fb, sc))

                def VP(xt, c2, sb):
                    return ("P", lambda: v_pass(xt, c2, sb))

                def Ao(pr, q2, kbs):
                    return [("A", pr, q2, kb) for kb in kbs]

                vp0 = [VP(xv0, 0, sb) for sb in range(8)]
                vp1 = [VP(xv1, 1, sb) for sb in range(8)]

                # A-list ladder: each (pr, q2, kb) A-op behind the proj
                # passes that produce its kt/qt slices
                alist = []
                alist += [KQ("wk", kt, xk0, 0, 0, 0), KQ("wq", qt, xq0, 0, 0, 0)]
                alist += Ao(0, 0, range(0, 4))
                alist += [KQ("wk", kt, xk0, 0, 0, 1)] + Ao(0, 0, range(4, 8))
                alist += [KQ("wq", qt, xq0, 0, 0, 1)] + Ao(0, 1, range(0, 4))
                alist += [KQ("wk", kt, xk0, 0, 1, 0), KQ("wq", qt, xq0, 0, 1, 0)]
                for i, a in enumerate(Ao(0, 1, range(4, 8)) + Ao(1, 0, range(0, 4))):
                    alist += [a, vp0[i]]
                alist += [KQ("wk", kt, xk0, 0, 1, 1), KQ("wq", qt, xq0, 0, 1, 1)]
                alist += Ao(1, 0, range(4, 8)) + Ao(1, 1, range(0, 8))
                alist += [KQ("wk", kt, xk1, 1, 0, 0), KQ("wk", kt, xk1, 1, 0, 1)]
                alist += Ao(0, 0, range(8, 12)) + Ao(0, 0, range(12, 16))
                for i, a in enumerate(Ao(0, 1, range(8, 16))):
                    alist += [a, vp1[i]]
                alist += [KQ("wk", kt, xk1, 1, 1, 0), KQ("wk", kt, xk1, 1, 1, 1)]
                alist += Ao(1, 0, range(8, 16))
                alist += Ao(1, 1, range(8, 16))
                alist += [KQ("wq", qt, xq1, 1, 0, 0)] + Ao(0, 2, range(NKB))
                alist += [KQ("wq", qt, xq1, 1, 1, 0)] + Ao(1, 2, range(NKB))
                alist += [KQ("wq", qt, xq1, 1, 0, 1)] + Ao(0, 3, range(NKB))
                alist += [KQ("wq", qt, xq1, 1, 1, 1)] + Ao(1, 3, range(NKB))

                # positions for dependency gating
                apos = {}
                vp_last = {0: 0, 1: 0}
                for i, op in enumerate(alist):
                    if op[0] == "A":
                        apos[op[1:]] = i
                for i, op in enumerate(alist):
                    if op[0] == "P" and i < len(alist):
                        pass
                # vx halves: find index after last vp0/vp1 in alist
                idx = 0
                for i, op in enumerate(alist):
                    if op is vp0[-1]:
                        vp_last[0] = i
                    if op is vp1[-1]:
                        vp_last[1] = i

                # B-FIFO in psum-pool order: (min_a_index, op)
                border = [(0, 0), (0, 1), (1, 0), (1, 1),
                          (0, 2), (1, 2), (0, 3), (1, 3)]
                bfifo = []
                for pr, q2 in border:
                    for kb in range(NKB):
                        gate = apos[(pr, q2, kb)] + 3
                        gate = max(gate, vp_last[0 if kb < 8 else 1] + 1)
                        bfifo.append((gate, lambda pr=pr, q2=q2, kb=kb:
                                      B_iter(pr, q2, kb)))
                    bfifo.append((apos[(pr, q2, NKB - 1)] + 3,
                                  lambda pr=pr, q2=q2: B_tail(pr, q2)))

                # emit: walk the A-list, draining the B-FIFO proportionally
                total = len(alist)
                nb = len(bfifo)
                bi = 0
                for i, op in enumerate(alist):
                    if op[0] == "A":
                        A_iter(op[1], op[2], op[3])
                    else:
                        op[1]()
                    want = nb * (i + 1) // total + 2
                    while bi < min(want, nb) and bfifo[bi][0] <= i:
                        bfifo[bi][1]()
                        bi += 1
                while bi < nb:
                    bfifo[bi][1]()
                    bi += 1

    nc.finalize()
    return nc


_PROG_CACHE = {}


def _get_program(use_mask):
    key = bool(use_mask)
    if key not in _PROG_CACHE:
        _PROG_CACHE[key] = build_program(key)
    return _PROG_CACHE[key]


def _warr(w):
    """[1024, f] -> [128, dt*f] partition-major (contiguous device DMA)."""
    f = w.shape[1]
    return np.ascontiguousarray(
        w.reshape(NDT, 128, f).transpose(1, 0, 2).reshape(128, NDT * f))


def kernel(query, key, value, key_mask, Wq, Wk, Wv, bias_table):
    import ml_dtypes
    bf16 = ml_dtypes.bfloat16
    f16 = np.float16

    query = np.asarray(query, dtype=np.float32)
    key = np.asarray(key, dtype=np.float32)
    value = np.asarray(value, dtype=np.float32)
    key_mask = np.asarray(key_mask, dtype=np.float32)
    Wq = np.asarray(Wq, dtype=np.float32)
    Wk = np.asarray(Wk, dtype=np.float32)
    Wv = np.asarray(Wv, dtype=np.float32)
    bias_table = np.asarray(bias_table, dtype=np.float32)

    use_mask = not np.all(key_mask == 1.0)
    nc = _get_program(use_mask)

    buckets = _rel_buckets()  # [2S-1] for rel = k-q in [-(S-1), S-1]
    g = bias_table[buckets]   # [2S-1, H] bias as function of rel
    in_maps = []
    for core in range(NCORES):
        b, hg = core // 4, core % 4
        hsl = slice(hg * HPC * HD, (hg + 1) * HPC * HD)
        heads = np.arange(hg * HPC, (hg + 1) * HPC)
        c31 = bias_table[31, heads]  # rel >= +128
        c15 = bias_table[15, heads]  # rel <= -128
        cmaj = np.stack([c31, c15])               # [side, h]
        cmin = np.stack([c15, c31])
        # -32 keeps the unnormalized exps in a sane fp32 range (softmax is
        # shift-invariant; numerator and denominator scale together)
        cv = np.stack([cmaj - 32.0, np.exp(cmin - cmaj)]).astype(np.float32)
        # band tables: ebt[side, h, p, w] = exp(g_h(p - w + 128) - cmaj)
        p = np.arange(128)[:, None]
        w = np.arange(EBW)[None, :]
        rel = p - w + 128                          # in (-256, 256)
        gh = g[rel + (S - 1)][:, :, heads]         # [128, EBW, HPC]
        ebt_np = np.empty((2, HPC, 128, EBW), np.float32)
        for mi in range(2):
            ebt_np[mi] = np.exp(
                gh - cmaj[mi][None, None, :]).transpose(2, 0, 1)
        im = {
            "xv": np.ascontiguousarray(value[b].T).astype(f16),
            "xq": np.ascontiguousarray(query[b].T).astype(f16),
            "xk": np.ascontiguousarray(key[b].T).astype(f16),
            "wq": _warr(Wq[:, hsl]).astype(f16),
            "wk": _warr(Wk[:, hsl]).astype(f16),
            "wv": _warr(Wv[:, hsl]).astype(f16),
            "ebt": np.ascontiguousarray(
                ebt_np.transpose(2, 0, 1, 3).reshape(128, -1)).astype(bf16),
            "cvals": np.broadcast_to(cv, (128,) + cv.shape).copy(),
        }
        if use_mask:
            madd = (-1e4 * (1.0 - key_mask[b])).astype(np.float32)
            im["mvals"] = np.ascontiguousarray(madd.reshape(NKB, 128).T)
        in_maps.append(im)

    res = run_bass_kernel_spmd(nc, in_maps, core_ids=list(range(NCORES)))
    out = np.empty((B, S, H * HD), np.float32)
    for core in range(NCORES):
        b, hg = core // 4, core % 4
        o = res.results[core]["out"]  # [HPC, HD+1, S]; row 0 = denominators
        for h in range(HPC):
            out[b, :, (hg * HPC + h) * HD:(hg * HPC + h + 1) * HD] = \
                (o[h, 1:] / o[h, 0:1]).T
    return out


# revision 24
# speedup vs baseline: 1.0719x; 1.0324x over previous
"""T5-style multi-head attention on 8 Trainium2 NeuronCores.

Problem: B=2, S=2048, D=1024, H=16 heads of 64; T5 relative-position bias
(32 buckets, max_distance=128), key mask, softmax, context.

Sharding: data-parallel over B (2) x tensor-parallel over head-groups of 4
(4 groups) = 8 cores.  Each core computes Q/K/V projections for its batch
and its 4 heads, then full attention for those heads.

v2: bf16 inputs/weights/activations (halves the input-DMA volume and
enables fast-weight-load), column-streamed projections software-pipelined
with the attention phase so the ACT engine (exp is the per-core compute
floor at ~126us) starts ~20us in instead of after all projections.

Device algorithm (per core), matmul cycles at 1 col/cycle in bf16:
  stream order: xk/xq/xv column-chunks of 1024; projections consume each
  chunk as it lands (Q/K as (X W)^T in [f, s] layout, V in [s, d] layout
  with a ones column for the softmax denominator).
  attention per (head, q2 chunk of 1024, k block of 128):
    scoresT[k,q] = K^T.T Q^T   (contraction d=64)
    expS = exp(scoresT + c_maj - 32) on ACT (c_maj = saturated-bucket bias)
    band fix (DVE) + minority saturated side (GPSIMD) as multiplicative
    corrections; ctxT[d|1,q] += V_ext.T expS accumulated over k blocks;
    row 0 of ctxT = softmax denominators (ones column rides along free).
  tail: reciprocal (DVE), partition broadcast (GPSIMD), scale, DMA out.

The first head's first-half k blocks are emitted before the second half
of the K/V streams arrive so ACT has work during the stream tail; its es
tiles for head h1 are held in SBUF until V lands.
"""

import numpy as np

import concourse.bacc as bacc
import concourse.tile as tile
from concourse import mybir
from concourse.bass_utils import run_bass_kernel_spmd

# problem dims (hardcoded per contract)
B = 2
S = 2048
DM = 1024
H = 16
HD = 64
NB = 32
MAXD = 128

HPC = 4          # heads per core
NCORES = 8
NDT = DM // 128  # 8 contraction tiles
NKB = S // 128   # 16 k blocks
NQ2 = 2          # q windows of 1024
QW = 1024        # q window width
CW = 1024        # input stream chunk width
EBW = 384        # band table width

F32 = mybir.dt.float32
F32R = mybir.dt.float32r
BF16 = mybir.dt.bfloat16
F16 = mybir.dt.float16


def _rel_buckets():
    """T5 bidirectional bucket for rel = k - q in [-(S-1), S-1], fp32 math."""
    rel = np.arange(-(S - 1), S, dtype=np.int64)
    nb = NB // 2
    ret = (rel > 0).astype(np.int64) * nb
    rp = np.abs(rel)
    max_exact = nb // 2
    is_small = rp < max_exact
    rp_f = np.maximum(rp, 1).astype(np.float32)
    val = np.log(rp_f / np.float32(max_exact)) / np.float32(
        np.log(MAXD / max_exact)
    ) * np.float32(nb - max_exact)
    # XLA CPU f32->s32 convert rounds to nearest (cvtps2dq), not truncates
    val_large = max_exact + np.rint(val).astype(np.int32)
    val_large = np.minimum(val_large, nb - 1)
    return (ret + np.where(is_small, rp, val_large)).astype(np.int64)  # [2S-1]


def _band_bounds(kb):
    """Columns [a,b) of the non-saturated diagonal band for k block kb."""
    a = max(0, (kb - 1) * 128)
    b = min(S, (kb + 2) * 128)
    return a, b


def _maj_side(kb, q2):
    """Majority saturated side for (k block, q chunk): 0 -> bucket31 (q<a),
    1 -> bucket15 (q>=b)."""
    qlo, qhi = q2 * QW, (q2 + 1) * QW
    a, b = _band_bounds(kb)
    len31 = max(0, min(qhi, a) - qlo)
    len15 = max(0, qhi - max(qlo, b))
    return 0 if len31 >= len15 else 1


def build_program(use_mask, reps=1):
    nc = bacc.Bacc("TRN2", target_bir_lowering=False, debug=False,
                   num_devices=NCORES)

    xv = nc.dram_tensor("xv", [DM, S], F16, kind="ExternalInput").ap()
    xq = nc.dram_tensor("xq", [DM, S], F16, kind="ExternalInput").ap()
    xk = nc.dram_tensor("xk", [DM, S], F16, kind="ExternalInput").ap()
    # weights pre-arranged host-side to [128, dt, f] so the load is one
    # contiguous-descriptor DMA (gathers here would steal DMA engines from
    # the startup-critical x stream)
    wq = nc.dram_tensor("wq", [128, NDT * HPC * HD], F16,
                        kind="ExternalInput").ap()
    wk = nc.dram_tensor("wk", [128, NDT * HPC * HD], F16,
                        kind="ExternalInput").ap()
    wv = nc.dram_tensor("wv", [128, NDT * HPC * HD], F16,
                        kind="ExternalInput").ap()
    # band tables exp(g_h(rel) - c_maj), pre-arranged to partition-major
    ebt = nc.dram_tensor("ebt", [128, 2 * HPC * EBW], BF16,
                         kind="ExternalInput").ap()
    # per-(side, head): exp bias constant c_maj and minority ratio
    # cvals[0, side, h] = c_maj - 32 ; cvals[1, side, h] = exp(c_min - c_maj)
    cvals = nc.dram_tensor("cvals", [128, 2, 2, HPC], F32,
                           kind="ExternalInput").ap()
    if use_mask:
        # additive mask term -1e4*(1-mask) laid out [128, NKB]
        mvals = nc.dram_tensor("mvals", [128, NKB], F32,
                               kind="ExternalInput").ap()
    # row 0 = softmax denominators, rows 1..64 = unnormalized context;
    # the division happens host-side during unsharding
    outp = nc.dram_tensor("out", [HPC, HD + 1, S], F32,
                          kind="ExternalOutput").ap()

    with tile.TileContext(nc) as tc:
        with tc.tile_pool(name="const", bufs=1) as const, \
             tc.tile_pool(name="qkt", bufs=1) as qkt, \
             tc.tile_pool(name="xs", bufs=3) as xs, \
             tc.tile_pool(name="esp", bufs=40) as esp, \
             tc.tile_pool(name="stgp", bufs=3) as stgp:

            # ---- resident constants ----
            w_sb = {}
            for nm, src in (("wk", wk), ("wq", wq), ("wv", wv)):
                t = const.tile([128, NDT, HPC * HD], F16, tag=nm, name=nm)
                nc.gpsimd.dma_start(
                    out=t[:], in_=src.rearrange("p (dt f) -> p dt f", dt=NDT))
                w_sb[nm] = t
            cb = const.tile([128, 2, 2, HPC], F32, tag="cb", name="cb")
            nc.gpsimd.dma_start(out=cb[:], in_=cvals[:])
            eb_sb = const.tile([128, 2, HPC, EBW], BF16, tag="eb", name="eb")
            nc.gpsimd.dma_start(
                out=eb_sb[:],
                in_=ebt.rearrange("p (m h w) -> p m h w", m=2, h=HPC))
            if use_mask:
                mk = const.tile([128, NKB], F32, tag="mk", name="mk")
                nc.gpsimd.dma_start(out=mk[:], in_=mvals[:])

            # Q^T/K^T per pair: [128(2 heads x 64d), S] bf16
            qt = [qkt.tile([128, S], F16, tag=f"qt{p}", name=f"qt{p}")
                  for p in range(2)]
            kt = [qkt.tile([128, S], F16, tag=f"kt{p}", name=f"kt{p}")
                  for p in range(2)]
            # V_ext: [128(k in block), head, kblock, 65(1|d)]
            vx = qkt.tile([128, HPC, NKB, HD + 1], BF16, tag="vx", name="vx")
            nc.vector.memset(vx[:], 1.0)

            for _rep in range(reps):
              # psum pools: sps/proj 3x[128,1024] (6 banks) +
              # ctx 1x[65,1024] (2 banks) = 8 banks
              with tc.tile_pool(name="spsp", bufs=3, space="PSUM") as spsp, \
                   tc.tile_pool(name="ctxp", bufs=1, space="PSUM") as ctxp:

                def stream_chunk(src, tag, c2, q=None):
                    """One [dm, 1024]-column chunk as 4 DMAs of dt-pairs
                    (few issues, 2KB descriptors, fine-grained deps)."""
                    q = q or nc.sync
                    t = xs.tile([128, NDT, CW], F16, tag="x", name=f"x{tag}")
                    for dp in range(4):
                        q.dma_start(
                            out=t[:, 2 * dp:2 * dp + 2, :],
                            in_=src[256 * dp:256 * (dp + 1),
                                    c2 * CW:(c2 + 1) * CW].rearrange(
                                "(dt p) w -> p dt w", p=128))
                    return t

                def qk_pass(wname, dst, xt, c2, fb):
                    """One 1024-col psum pass of (X W)^T."""
                    ps = spsp.tile([128, CW], F32, tag="s", name="pj")
                    for dt in range(NDT):
                        for sc in range(2):
                            nc.tensor.matmul(
                                ps[:, sc * 512:(sc + 1) * 512],
                                lhsT=w_sb[wname][:, dt,
                                                 fb * 128:(fb + 1) * 128],
                                rhs=xt[:, dt, sc * 512:(sc + 1) * 512],
                                start=(dt == 0), stop=(dt == NDT - 1))
                    nc.vector.tensor_copy(
                        out=dst[fb][:, c2 * CW:(c2 + 1) * CW], in_=ps[:])

                def v_pass(xt, c2, sbp):
                    """V[s,d] for an s-block pair (cols 0-255 of each
                    bank of a 2-bank tile)."""
                    ps = spsp.tile([128, CW], F32, tag="s", name="pv")
                    for dt in range(NDT):
                        for j in range(2):
                            sb = sbp * 2 + j
                            nc.tensor.matmul(
                                ps[:, j * 512:j * 512 + 256],
                                lhsT=xt[:, dt, sb * 128:(sb + 1) * 128],
                                rhs=w_sb["wv"][:, dt, :],
                                start=(dt == 0), stop=(dt == NDT - 1))
                    for j in range(2):
                        nc.vector.tensor_copy(
                            out=vx[:, :, c2 * 8 + sbp * 2 + j, 1:HD + 1],
                            in_=ps[:, j * 512:j * 512 + 256].rearrange(
                                "p (h d) -> p h d", h=HPC))

                def score_mms(sps, h, q2, kb):
                    pr, hl = h // 2, h % 2
                    ksl = kt[pr][hl * 64:(hl + 1) * 64,
                                 kb * 128:(kb + 1) * 128]
                    for hf in range(2):
                        qsl = qt[pr][hl * 64:(hl + 1) * 64,
                                     q2 * QW + hf * 512:
                                     q2 * QW + (hf + 1) * 512]
                        nc.tensor.matmul(
                            sps[:, hf * 512:(hf + 1) * 512],
                            lhsT=ksl, rhs=qsl, start=True, stop=True)

                def exp_fix(sps, h, q2, kb):
                    """exp + region fixes -> es tile (SBUF bf16)."""
                    if use_mask:
                        nc.vector.tensor_scalar_add(
                            sps[:], sps[:], mk[:, kb:kb + 1])
                    mi = _maj_side(kb, q2)
                    es = esp.tile([128, QW], BF16, tag="es", name="es")
                    nc.scalar.activation(
                        out=es[:], in_=sps[:],
                        func=mybir.ActivationFunctionType.Exp,
                        bias=cb[:, 0, mi, h:h + 1], scale=1.0)
                    # band fix + minority saturated side, both on DVE
                    a, b = _band_bounds(kb)
                    qlo = q2 * QW
                    bs, be = max(qlo, a), min(qlo + QW, b)
                    if bs < be:
                        w0 = bs - (kb - 1) * 128
                        nc.vector.tensor_mul(
                            es[:, bs - qlo:be - qlo],
                            es[:, bs - qlo:be - qlo],
                            eb_sb[:, mi, h, w0:w0 + (be - bs)])
                    if mi == 0:
                        ms, me = max(qlo, b), qlo + QW
                    else:
                        ms, me = qlo, min(qlo + QW, a)
                    if ms < me:
                        nc.vector.tensor_scalar_mul(
                            es[:, ms - qlo:me - qlo],
                            es[:, ms - qlo:me - qlo],
                            cb[:, 1, mi, h:h + 1])
                    return es

                def attn_ctx(ctx, h, kb, es):
                    for hf in range(2):
                        nc.tensor.matmul(
                            ctx[:, hf * 512:(hf + 1) * 512],
                            lhsT=vx[:, h, kb, :],
                            rhs=es[:, hf * 512:(hf + 1) * 512],
                            start=(kb == 0), stop=(kb == NKB - 1))

                def attn_tail(ctx, h, q2):
                    # evacuate psum; row 0 = softmax denominators
                    # (normalization happens host-side during unsharding)
                    stg = stgp.tile([HD + 1, QW], F32, tag="stg", name="stg")
                    nc.vector.tensor_copy(out=stg[:], in_=ctx[:])
                    nc.gpsimd.dma_start(
                        out=outp[h, :, q2 * QW:(q2 + 1) * QW],
                        in_=stg[:])

                def pair_block(pr, q2, kbs, ctxs=None, held=None,
                               scores_only=False):
                    """One (head-pair, q chunk) span of k blocks.

                    The two heads' score matmuls are emitted back-to-back:
                    their contraction rows are disjoint 64-row groups, so
                    the PE runs them concurrently (row tiling).
                    """
                    h0, h1 = 2 * pr, 2 * pr + 1
                    for kb in kbs:
                        for i, h in enumerate((h0, h1)):
                            es = held.pop((h, kb), None) if held else None
                            if es is None and not scores_only:
                                es = attn_scores(h, q2, kb)
                            elif es is None:
                                held[(h, kb)] = attn_scores(h, q2, kb)
                                continue
                            attn_ctx(ctxs[i], h, kb, es)

                def pair_tail(ctxs, pr, q2):
                    attn_tail(ctxs[0], 2 * pr, q2)
                    attn_tail(ctxs[1], 2 * pr + 1, q2)

                def ctx_pair():
                    return [ctxp.tile([HD + 1, QW], F32, tag="ctx",
                                      name="ctx") for _ in range(2)]

                # ---- streaming + overlapped schedule ----
                # A-ops: one head PAIR's scores (row-group-paired matmuls)
                # + two 1024-wide exps; es held in SBUF.  B-ops: per-head
                # ctx accumulation blocks draining a FIFO in psum-pool
                # order.  Projection passes are embedded in the A-ladder at
                # their data-arrival points.
                held = {}
                ctx_open = {}

                def A_iter(pr, q2, kb):
                    h0, h1 = 2 * pr, 2 * pr + 1
                    sps0 = spsp.tile([128, QW], F32, tag="s", name="s")
                    sps1 = spsp.tile([128, QW], F32, tag="s", name="s")
                    score_mms(sps0, h0, q2, kb)
                    score_mms(sps1, h1, q2, kb)
                    held[(h0, q2, kb)] = exp_fix(sps0, h0, q2, kb)
                    held[(h1, q2, kb)] = exp_fix(sps1, h1, q2, kb)

                def B_iter(h, q2, kb):
                    key = (h, q2)
                    if key not in ctx_open:
                        ctx_open[key] = ctxp.tile([HD + 1, QW], F32,
                                                  tag="ctx", name="ctx")
                    attn_ctx(ctx_open[key], h, kb, held.pop((h, q2, kb)))

                def B_tail(h, q2):
                    attn_tail(ctx_open.pop((h, q2)), h, q2)

                xk0 = stream_chunk(xk, "k", 0)
                xq0 = stream_chunk(xq, "q", 0)
                xv0 = stream_chunk(xv, "v", 0, q=nc.gpsimd)
                xv1 = stream_chunk(xv, "v", 1, q=nc.gpsimd)
                xk1 = stream_chunk(xk, "k", 1)
                xq1 = stream_chunk(xq, "q", 1)

                def KQ(w, dst, xt, c2, fb):
                    return ("P", lambda: qk_pass(w, dst, xt, c2, fb))

                def VP(xt, c2, sbp):
                    return ("P", lambda: v_pass(xt, c2, sbp))

                def Ao(pr, q2, kbs):
                    return [("A", pr, q2, kb) for kb in kbs]

                vp0 = [VP(xv0, 0, sbp) for sbp in range(4)]
                vp1 = [VP(xv1, 1, sbp) for sbp in range(4)]

                alist = []
                alist += [KQ("wk", kt, xk0, 0, 0), KQ("wq", qt, xq0, 0, 0)]
                alist += Ao(0, 0, range(0, 8))
                alist += [KQ("wk", kt, xk0, 0, 1), KQ("wq", qt, xq0, 0, 1)]
                for i, a in enumerate(Ao(1, 0, range(0, 8))):
                    alist += [a] + ([vp0[i // 2]] if i % 2 == 0 else [])
                alist += [KQ("wk", kt, xk1, 1, 0)]
                for i, a in enumerate(Ao(0, 0, range(8, 16))):
                    alist += [a] + ([vp1[i // 2]] if i % 2 == 0 else [])
                alist += [KQ("wk", kt, xk1, 1, 1)]
                alist += Ao(1, 0, range(8, 16))
                alist += [KQ("wq", qt, xq1, 1, 0)] + Ao(0, 1, range(NKB))
                alist += [KQ("wq", qt, xq1, 1, 1)] + Ao(1, 1, range(NKB))

                apos = {}
                vp_last = {0: 0, 1: 0}
                for i, op in enumerate(alist):
                    if op[0] == "A":
                        apos[op[1:]] = i
                    if op in vp0:
                        vp_last[0] = i
                    if op in vp1:
                        vp_last[1] = i

                # per-head B blocks in psum-pool order
                border = [(0, 0), (1, 0), (2, 0), (3, 0),
                          (0, 1), (1, 1), (2, 1), (3, 1)]
                bfifo = []
                for h, q2 in border:
                    pr = h // 2
                    for kb in range(NKB):
                        gate = apos[(pr, q2, kb)] + 2
                        gate = max(gate, vp_last[0 if kb < 8 else 1] + 1)
                        bfifo.append((gate, lambda h=h, q2=q2, kb=kb:
                                      B_iter(h, q2, kb)))
                    bfifo.append((apos[(pr, q2, NKB - 1)] + 2,
                                  lambda h=h, q2=q2: B_tail(h, q2)))

                total = len(alist)
                nb = len(bfifo)
                bi = 0
                for i, op in enumerate(alist):
                    if op[0] == "A":
                        A_iter(op[1], op[2], op[3])
                    else:
                        op[1]()
                    want = nb * (i + 1) // total + 4
                    while bi < min(want, nb) and bfifo[bi][0] <= i:
                        bfifo[bi][1]()
                        bi += 1
                while bi < nb:
                    bfifo[bi][1]()
                    bi += 1

    nc.finalize()
    return nc


_PROG_CACHE = {}


def _get_program(use_mask):
    key = bool(use_mask)
    if key not in _PROG_CACHE:
        _PROG_CACHE[key] = build_program(key)
    return _PROG_CACHE[key]


def _warr(w):
    """[1024, f] -> [128, dt*f] partition-major (contiguous device DMA)."""
    f = w.shape[1]
    return np.ascontiguousarray(
        w.reshape(NDT, 128, f).transpose(1, 0, 2).reshape(128, NDT * f))


def kernel(query, key, value, key_mask, Wq, Wk, Wv, bias_table):
    import ml_dtypes
    bf16 = ml_dtypes.bfloat16
    f16 = np.float16

    query = np.asarray(query, dtype=np.float32)
    key = np.asarray(key, dtype=np.float32)
    value = np.asarray(value, dtype=np.float32)
    key_mask = np.asarray(key_mask, dtype=np.float32)
    Wq = np.asarray(Wq, dtype=np.float32)
    Wk = np.asarray(Wk, dtype=np.float32)
    Wv = np.asarray(Wv, dtype=np.float32)
    bias_table = np.asarray(bias_table, dtype=np.float32)

    use_mask = not np.all(key_mask == 1.0)
    nc = _get_program(use_mask)

    buckets = _rel_buckets()  # [2S-1] for rel = k-q in [-(S-1), S-1]
    g = bias_table[buckets]   # [2S-1, H] bias as function of rel
    in_maps = []
    for core in range(NCORES):
        b, hg = core // 4, core % 4
        hsl = slice(hg * HPC * HD, (hg + 1) * HPC * HD)
        heads = np.arange(hg * HPC, (hg + 1) * HPC)
        c31 = bias_table[31, heads]  # rel >= +128
        c15 = bias_table[15, heads]  # rel <= -128
        cmaj = np.stack([c31, c15])               # [side, h]
        cmin = np.stack([c15, c31])
        # -32 keeps the unnormalized exps in a sane fp32 range (softmax is
        # shift-invariant; numerator and denominator scale together)
        cv = np.stack([cmaj - 32.0, np.exp(cmin - cmaj)]).astype(np.float32)
        # band tables: ebt[side, h, p, w] = exp(g_h(p - w + 128) - cmaj)
        p = np.arange(128)[:, None]
        w = np.arange(EBW)[None, :]
        rel = p - w + 128                          # in (-256, 256)
        gh = g[rel + (S - 1)][:, :, heads]         # [128, EBW, HPC]
        ebt_np = np.empty((2, HPC, 128, EBW), np.float32)
        for mi in range(2):
            ebt_np[mi] = np.exp(
                gh - cmaj[mi][None, None, :]).transpose(2, 0, 1)
        im = {
            "xv": np.ascontiguousarray(value[b].T).astype(f16),
            "xq": np.ascontiguousarray(query[b].T).astype(f16),
            "xk": np.ascontiguousarray(key[b].T).astype(f16),
            "wq": _warr(Wq[:, hsl]).astype(f16),
            "wk": _warr(Wk[:, hsl]).astype(f16),
            "wv": _warr(Wv[:, hsl]).astype(f16),
            "ebt": np.ascontiguousarray(
                ebt_np.transpose(2, 0, 1, 3).reshape(128, -1)).astype(bf16),
            "cvals": np.broadcast_to(cv, (128,) + cv.shape).copy(),
        }
        if use_mask:
            madd = (-1e4 * (1.0 - key_mask[b])).astype(np.float32)
            im["mvals"] = np.ascontiguousarray(madd.reshape(NKB, 128).T)
        in_maps.append(im)

    res = run_bass_kernel_spmd(nc, in_maps, core_ids=list(range(NCORES)))
    out = np.empty((B, S, H * HD), np.float32)
    for core in range(NCORES):
        b, hg = core // 4, core % 4
        o = res.results[core]["out"]  # [HPC, HD+1, S]; row 0 = denominators
        for h in range(HPC):
            out[b, :, (hg * HPC + h) * HD:(hg * HPC + h + 1) * HD] = \
                (o[h, 1:] / o[h, 0:1]).T
    return out
